# revision 65
# baseline (speedup 1.0000x reference)
"""LocalAttention1d Trainium2 kernel (v5).

Layout strategy (B=16 sharded over 8 cores, 2 batches/core), processed as
4 pipeline units of 512 t-rows (half-batches) to keep the PE queue dense:

  - p_t chain in ~fp32 precision: h = tanh(c@W_p.T) via fp16x2 split matmuls
    (3 cross terms); logit = <tanh(h), V_p> via DVE STT accumulate.
  - Window rows p_int-3..p_int+3 are contiguous in q^T; one SWDGE descriptor
    per t gathers the whole 7 KB window via an overlapping strided DRAM view
    (elem_size=3584, elem_step=512).
  - scores per tile: ONE broadcast product (DVE) over [128, 7, 512] + ONE
    segmented tensor_reduce -> a[128, 7].
  - softmax kept unnormalized: wt = exp(a - max) * gauss; the 1/sum scale is
    folded into the PSUM->SBUF output copy (ACT per-partition scale), so the
    diag-weight build (GPS) does not wait on the reciprocal.
  - weighted sum: 7 diagonal fp16 matmuls accumulate in PSUM; output stored
    fp16, widened on host.
  - Emission orders each engine queue so the PE stream
    h(u0) h(u1) u(u0) h(u2) u(u1) w(u0) h(u3) u(u2) w(u1) u(u3) w(u2) w(u3)
    never stalls; c_t is DMAd in per-unit chunks so the first matmul starts
    ~3 us in.
"""

import sys

sys.path.insert(0, "/opt/trn_rl_repo")

import numpy as np

import concourse.bass as bass
import concourse.tile as tile
from concourse import bacc, mybir
from concourse.bass_utils import run_bass_kernel_spmd

B, T, S, QS, CS, PS, D = 16, 1024, 4096, 512, 512, 512, 3
NCORE = 8
BPC = B // NCORE
NJ = 2 * D + 1
NU = 4          # pipeline units per core (half-batches)
NTU = 4         # 128-row tiles per unit
HT = 512        # t-rows per unit
WIN = NJ * QS
NWU = NTU * NJ  # score cols per unit (28)

dt = mybir.dt
AF = mybir.ActivationFunctionType
ALU = mybir.AluOpType
AX = mybir.AxisListType

LAST_EXEC_NS = None
LAST_RESULT = None
_CACHE = {}


def _build_nc():
    nc = bacc.Bacc("TRN2", target_bir_lowering=False, debug=False, num_devices=NCORE)

    qT16_h = nc.dram_tensor("qT16", [BPC, S, QS], dt.float16, kind="ExternalInput")
    # c chunks pre-arranged on host: [batch, half, partition, k, n] so each
    # partition's DMA run is 4 KB contiguous
    cT1 = nc.dram_tensor("cT1", [BPC, 2, 128, 4, HT], dt.float16, kind="ExternalInput").ap()
    cT2 = nc.dram_tensor("cT2", [BPC, 2, 128, 4, HT], dt.float16, kind="ExternalInput").ap()
    wp1 = nc.dram_tensor("wp1", [CS, PS], dt.float16, kind="ExternalInput").ap()
    wp2 = nc.dram_tensor("wp2", [CS, PS], dt.float16, kind="ExternalInput").ap()
    wa1 = nc.dram_tensor("wa1", [CS, QS], dt.float16, kind="ExternalInput").ap()
    vpr = nc.dram_tensor("vpr", [128, PS], dt.float32, kind="ExternalInput").ap()
    offs = nc.dram_tensor("offs", [128, NWU], dt.float32, kind="ExternalInput").ap()
    perm8 = nc.dram_tensor("perm8", [128, 8, 128], dt.float32, kind="ExternalInput").ap()
    id128h = nc.dram_tensor("id128h", [128, 128], dt.float16, kind="ExternalInput").ap()
    out = nc.dram_tensor("out", [BPC, T, QS], dt.float16, kind="ExternalOutput").ap()

    with tile.TileContext(nc) as tc:
        import contextlib

        ctx = contextlib.ExitStack()
        with ctx:
            cpool = ctx.enter_context(tc.tile_pool(name="consts", bufs=1))
            ctp = ctx.enter_context(tc.tile_pool(name="ct", bufs=2))
            up = ctx.enter_context(tc.tile_pool(name="u16", bufs=8))
            pp = ctx.enter_context(tc.tile_pool(name="prod", bufs=2))
            sp = ctx.enter_context(tc.tile_pool(name="small", bufs=2))
            gpool = ctx.enter_context(tc.tile_pool(name="gt", bufs=3))
            jp = ctx.enter_context(tc.tile_pool(name="junk", bufs=2))
            op = ctx.enter_context(tc.tile_pool(name="outp", bufs=2))
            mmp = ctx.enter_context(tc.tile_pool(name="mm", bufs=2, space="PSUM"))
            ump = ctx.enter_context(tc.tile_pool(name="um", bufs=2, space="PSUM"))
            wsp = ctx.enter_context(tc.tile_pool(name="ws", bufs=2, space="PSUM"))
            tpp = ctx.enter_context(tc.tile_pool(name="tp", bufs=1, space="PSUM"))

            # ---- per-unit state ----
            ct1u = [None] * NU
            ct2u = [None] * NU
            logits = [None] * NU
            idxs = [None] * NU
            gts = [None] * NU
            u16s = [[None] * NTU for _ in range(NU)]
            a_all = [None] * NU
            gauss = [None] * NU
            wt16 = [None] * NU
            rinv = [None] * NU

            def load_c(u):
                bu, hu = u // 2, u % 2
                c1t = ctp.tile([128, 4, HT], dt.float16, tag="ct1", bufs=4)
                nc.sync.dma_start(c1t[:], cT1[bu, hu])
                ct1u[u] = c1t
                c2t = ctp.tile([128, 4, HT], dt.float16, tag="ct2", bufs=2)
                nc.sync.dma_start(c2t[:], cT2[bu, hu])
                ct2u[u] = c2t

            load_c(0)
            wp1t = cpool.tile([128, 4, PS], dt.float16)
            nc.sync.dma_start(wp1t[:], wp1[:].rearrange("(k p) n -> p k n", p=128))
            wp2t = cpool.tile([128, 4, PS], dt.float16)
            nc.sync.dma_start(wp2t[:], wp2[:].rearrange("(k p) n -> p k n", p=128))
            vprt = cpool.tile([128, PS], dt.float32)
            nc.sync.dma_start(vprt[:], vpr[:])
            load_c(1)
            wa1t = cpool.tile([128, 4, QS], dt.float16)
            nc.sync.dma_start(wa1t[:], wa1[:].rearrange("(k p) n -> p k n", p=128))
            perm8t = cpool.tile([128, 8, 128], dt.float32)
            nc.sync.dma_start(perm8t[:], perm8[:])
            offst = cpool.tile([128, NWU], dt.float32)
            nc.sync.dma_start(offst[:], offs[:])
            id128ht = cpool.tile([128, 128], dt.float16)
            nc.sync.dma_start(id128ht[:], id128h[:])
            load_c(2)
            load_c(3)

            for u in range(NU):
                logits_t = sp.tile([128, NTU], dt.float32, tag=f"logits{u}")
                idxs_t = sp.tile([128, NTU * 8], dt.int16, tag=f"idxs{u}")
                a_all_t = sp.tile([128, NWU], dt.float32, tag=f"a_all{u}")
                wt16_t = sp.tile([128, NWU], dt.float16, tag=f"wt16{u}")
                rinv_t = sp.tile([128, NTU], dt.float32, tag=f"rinv{u}")
                logits[u], idxs[u], a_all[u] = logits_t, idxs_t, a_all_t
                wt16[u], rinv[u] = wt16_t, rinv_t

            def h_tile(u, m):
                """12 fp16x2 matmuls + tanh; logit = <tanh, V_p> via GPS
                product + ACT copy-accumulate (keeps the idx chain off DVE)."""
                hps = mmp.tile([128, PS], dt.float32, tag="hps", space="PSUM")
                nmm = 0
                for k in range(4):
                    lhs1 = ct1u[u][:, k, m * 128 : (m + 1) * 128]
                    lhs2 = ct2u[u][:, k, m * 128 : (m + 1) * 128]
                    for lhs, rhs in (
                        (lhs1, wp1t[:, k, :]),
                        (lhs1, wp2t[:, k, :]),
                        (lhs2, wp1t[:, k, :]),
                    ):
                        nc.tensor.matmul(hps[:], lhs, rhs, start=(nmm == 0), stop=(nmm == 11))
                        nmm += 1
                g = sp.tile([128, PS], dt.float32, tag="g", bufs=2)
                nc.scalar.activation(g[:], hps[:], AF.Tanh)
                junkf = jp.tile([128, PS], dt.float32, tag="junkf", bufs=1)
                nc.vector.scalar_tensor_tensor(
                    junkf[:], g[:], 1.0, vprt[:], ALU.bypass, ALU.mult,
                    accum_out=logits[u][:, m : m + 1],
                )

            def _floor(src, sfx, eng):
                shp = list(src[:].shape)
                i32 = sp.tile(shp, dt.int32, tag="fli" + sfx)
                eng.tensor_copy(i32[:], src[:])
                cand = sp.tile(shp, dt.float32, tag="flc" + sfx)
                eng.tensor_copy(cand[:], i32[:])
                corr = sp.tile(shp, dt.float32, tag="flx" + sfx)
                eng.tensor_tensor(corr[:], cand[:], src[:], ALU.is_gt)
                res = sp.tile(shp, dt.float32, tag="flr" + sfx)
                eng.tensor_tensor(res[:], cand[:], corr[:], ALU.subtract)
                return res

            p8s = [None] * NU
            pi8s = [None] * NU
            idxn = [None] * NU

            def idx_head(u):
                """p = S*sigmoid(l) via exp (avoids a 1.3us Sigmoid table
                reload on the idx critical path) + floor/clamp, all in the
                natural [128, m] layout; shared with the gauss path."""
                sig8 = sp.tile([128, NTU], dt.float32, tag="sig8")
                nc.scalar.activation(sig8[:], logits[u][:], AF.Sigmoid)
                p8 = sp.tile([128, NTU], dt.float32, tag=f"pt8_{u % 2}")
                nc.vector.tensor_scalar_mul(p8[:], sig8[:], 4096.0)
                pi8 = _floor(p8, f"8_{u % 2}", nc.vector)
                tmp = sp.tile([128, NTU], dt.float32, tag="tmpn")
                nc.vector.tensor_scalar(
                    tmp[:], pi8[:], 3.0, 0.0, ALU.subtract, ALU.max
                )
                ixn = sp.tile([128, NTU], dt.float32, tag="idxn")
                nc.vector.tensor_scalar(
                    ixn[:], tmp[:], float(S - NJ), None, ALU.min,
                )
                p8s[u], pi8s[u], idxn[u] = p8, pi8, ixn

            def perm_idx(u):
                """Permute natural-layout indices into the 16-partition-
                wrapped DGE layout (8 tiny PE matmuls + int16 copies)."""
                for w in range(8):
                    pps = tpp.tile([128, NTU], dt.float32, tag="pps", space="PSUM")
                    nc.tensor.matmul(
                        pps[:], perm8t[:, w, :], idxn[u][:, :],
                        start=True, stop=True,
                    )
                    nc.vector.tensor_copy(
                        idxs[u][:].rearrange("p (m w) -> p w m", w=8)[:, w, :],
                        pps[:],
                    )

            def gather(u):
                """Two half-unit gathers so the first tiles' data lands
                ~5us earlier than a single 512-descriptor gather."""
                bu = u // 2
                qwin = bass.AP(
                    tensor=qT16_h, offset=bu * S * QS,
                    ap=[[QS, S - NJ + 1], [1, WIN]],
                )
                gt = gpool.tile([128, NTU, WIN], dt.float16, tag="gt", bufs=3)
                nc.gpsimd.dma_gather(
                    gt[:], qwin, idxs[u][:, :],
                    NTU * 128, NTU * 128, WIN, elem_step=QS,
                    single_packet=False,
                )
                gts[u] = gt

            def gauss_path(u):
                p8, pi8 = p8s[u], pi8s[u]
                pos = sp.tile([128, NWU], dt.float32, tag="pos")
                pos3 = pos[:].rearrange("p (m j) -> p m j", j=NJ)
                nc.gpsimd.tensor_tensor(
                    pos3, pi8[:, :, None].broadcast_to([128, NTU, NJ]),
                    offst[:].rearrange("p (m j) -> p m j", j=NJ),
                    ALU.add,
                )
                dtile = sp.tile([128, NWU], dt.float32, tag="dtile")
                nc.gpsimd.tensor_tensor(
                    dtile[:].rearrange("p (m j) -> p m j", j=NJ),
                    p8[:, :, None].broadcast_to([128, NTU, NJ]),
                    pos3, ALU.subtract,
                )
                d2 = sp.tile([128, NWU], dt.float32, tag="d2")
                nc.gpsimd.tensor_tensor(d2[:], dtile[:], dtile[:], ALU.mult)
                gs = sp.tile([128, NWU], dt.float32, tag="gauss", bufs=3)
                nc.scalar.activation(gs[:], d2[:], AF.Exp, scale=float(-2.0 / 9.0))
                gauss[u] = gs

            def u_tile(u, m):
                ups = ump.tile([128, QS], dt.float32, tag="ups", space="PSUM")
                for k in range(4):
                    nc.tensor.matmul(
                        ups[:], ct1u[u][:, k, m * 128 : (m + 1) * 128],
                        wa1t[:, k, :], start=(k == 0), stop=(k == 3),
                    )
                if u16s[u][0] is None:
                    u16u = up.tile([128, NTU, QS], dt.float16, tag="u16", bufs=2)
                    for mm in range(NTU):
                        u16s[u][mm] = u16u
                nc.scalar.activation(u16s[u][m][:, m, :], ups[:], AF.Copy)

            def scores_tile_A(u, m):
                """DVE fold-chain: broadcast product, two fp16 pair-folds,
                short segmented reduce. All on DVE."""
                g3 = gts[u][:, m, :].rearrange("p (j q) -> p j q", j=NJ)
                prod = pp.tile([128, NJ, QS], dt.float16, tag="prod", bufs=1)
                nc.vector.tensor_tensor(
                    prod[:], g3,
                    u16s[u][m][:, m, None, :].broadcast_to([128, NJ, QS]),
                    ALU.mult,
                )
                f1 = pp.tile([128, NJ, QS // 2], dt.float16, tag="f1", bufs=1)
                nc.vector.tensor_tensor(
                    f1[:], prod[:, :, 0 : QS // 2], prod[:, :, QS // 2 : QS],
                    ALU.add,
                )
                f2 = pp.tile([128, NJ, QS // 4], dt.float16, tag="f2", bufs=1)
                nc.vector.tensor_tensor(
                    f2[:], f1[:, :, 0 : QS // 4], f1[:, :, QS // 4 : QS // 2],
                    ALU.add,
                )
                nc.vector.tensor_reduce(
                    a_all[u][:, m * NJ : (m + 1) * NJ][:, :, None],
                    f2[:], AX.X, ALU.add,
                )

            def scores_tile_B(u, m, eng):
                """eng product (DVE or GPS) + per-j ACT copy-accumulate."""
                gt = gts[u]
                prod = pp.tile([128, NJ, QS], dt.float16, tag="prodB", bufs=1)
                if eng is nc.vector:
                    g3 = gt[:, m, :].rearrange("p (j q) -> p j q", j=NJ)
                    eng.tensor_tensor(
                        prod[:], g3,
                        u16s[u][m][:, m, None, :].broadcast_to([128, NJ, QS]),
                        ALU.mult,
                    )
                else:
                    for j in range(NJ):
                        eng.tensor_tensor(
                            prod[:, j, :], gt[:, m, j * QS : (j + 1) * QS],
                            u16s[u][m][:, m, :], ALU.mult,
                        )
                for j in range(NJ):
                    junka = jp.tile([128, QS], dt.float16, tag="junk16")
                    nc.scalar.activation(
                        junka[:], prod[:, j, :], AF.Copy,
                        accum_out=a_all[u][:, m * NJ + j : m * NJ + j + 1],
                    )

            def softmax_part(u, mlo, mhi):
                nm = mhi - mlo
                sl = slice(mlo * NJ, mhi * NJ)
                a3 = a_all[u][:, sl].rearrange("p (m j) -> p m j", j=NJ)
                rmax = sp.tile([128, nm], dt.float32, tag=f"rmax{nm}")
                nc.vector.tensor_reduce(rmax[:, :, None], a3, AX.X, ALU.max)
                asub = sp.tile([128, nm * NJ], dt.float32, tag=f"asub{nm}")
                nc.vector.scalar_tensor_tensor(
                    asub[:].rearrange("p (m j) -> p m j", j=NJ),
                    rmax[:, :, None].broadcast_to([128, nm, NJ]), 1.0,
                    a3, ALU.bypass, ALU.subtract,
                )
                e_all = sp.tile([128, nm * NJ], dt.float32, tag=f"e_all{nm}")
                nc.scalar.activation(e_all[:], asub[:], AF.Exp, scale=-1.0)
                rsum = sp.tile([128, nm], dt.float32, tag=f"rsum{nm}")
                nc.vector.tensor_reduce(
                    rsum[:, :, None],
                    e_all[:].rearrange("p (m j) -> p m j", j=NJ), AX.X, ALU.add,
                )
                nc.vector.reciprocal(rinv[u][:, mlo:mhi], rsum[:])
                # unnormalized weights; 1/sum is applied at the output copy
                nc.vector.tensor_tensor(
                    wt16[u][:, sl], e_all[:], gauss[u][:, sl], ALU.mult
                )

            def softmax_unit(u):
                softmax_part(u, 0, NTU)

            dalls = {}

            def dall_tile(u, m):
                dall = sp.tile([128, NJ * 128], dt.float16, tag="dall", bufs=8)
                nc.vector.tensor_tensor(
                    dall[:].rearrange("p (j q) -> p j q", j=NJ),
                    id128ht[:, None, :].broadcast_to([128, NJ, 128]),
                    wt16[u][:, m * NJ : (m + 1) * NJ][:, :, None].broadcast_to(
                        [128, NJ, 128]
                    ),
                    ALU.mult,
                )
                dalls[(u, m)] = dall

            def wsum_tile(u, m):
                bu, hu = u // 2, u % 2
                gt = gts[u]
                dall = dalls[(u, m)]
                wps = wsp.tile([128, QS], dt.float32, tag="wps", space="PSUM")
                for j in range(NJ):
                    nc.tensor.matmul(
                        wps[:], dall[:, j * 128 : (j + 1) * 128],
                        gt[:, m, j * QS : (j + 1) * QS],
                        start=(j == 0), stop=(j == NJ - 1),
                    )
                outt = op.tile([128, QS], dt.float16, tag="outt")
                nc.scalar.activation(
                    outt[:], wps[:], AF.Copy, scale=rinv[u][:, m : m + 1]
                )
                t0 = hu * HT + m * 128
                nc.scalar.dma_start(out[bu, t0 : t0 + 128, :], outt[:])

            # ================= emission =================
            # PE stream per unit: h x4, u x4, perm, (wsum u-2).
            # DVE: score blocks of unit u-1 slotted between h logits.
            # GPS: idx chain -> gather desc first, then gauss smalls, dall.
            for u in range(NU):
                h_tile(u, 0)
                h_tile(u, 1)
                h_tile(u, 2)
                if u >= 1:
                    scores_tile_A(u - 1, 0)
                h_tile(u, 3)
                idx_head(u)
                for m in range(NTU):
                    u_tile(u, m)
                perm_idx(u)
                gather(u)
                if u >= 1:
                    scores_tile_A(u - 1, 1)
                    scores_tile_A(u - 1, 2)
                    scores_tile_B(u - 1, 3, nc.vector)
                gauss_path(u)
                if u >= 1:
                    softmax_unit(u - 1)
                    for m in range(NTU):
                        dall_tile(u - 1, m)
                if u >= 2:
                    for m in range(NTU):
                        wsum_tile(u - 2, m)
            uL = NU - 1
            scores_tile_A(uL, 0)
            scores_tile_A(uL, 1)
            for m in range(NTU):
                wsum_tile(NU - 2, m)
            softmax_part(uL, 0, 2)
            dall_tile(uL, 0)
            dall_tile(uL, 1)
            scores_tile_A(uL, 2)
            scores_tile_B(uL, 3, nc.vector)
            wsum_tile(uL, 0)
            wsum_tile(uL, 1)
            softmax_part(uL, 2, 4)
            dall_tile(uL, 2)
            dall_tile(uL, 3)
            wsum_tile(uL, 2)
            wsum_tile(uL, 3)

    nc.compile()
    return nc


def _host_prep(q, c_t, W_a, W_p, V_p):
    q = np.asarray(q, dtype=np.float32)
    c_t = np.asarray(c_t, dtype=np.float32)
    W_a = np.asarray(W_a, dtype=np.float32)
    W_p = np.asarray(W_p, dtype=np.float32)
    V_p = np.asarray(V_p, dtype=np.float32)

    qT16 = np.ascontiguousarray(q.transpose(0, 2, 1)).astype(np.float16)
    cT = np.ascontiguousarray(c_t.transpose(0, 2, 1))
    # [B, CS, T] -> [B, half, partition, k, n] with 4 KB contiguous runs
    cT = np.ascontiguousarray(
        cT.reshape(B, 4, 128, 2, HT).transpose(0, 3, 2, 1, 4)
    )
    cT1 = cT.astype(np.float16)
    cT2 = (cT - cT1.astype(np.float32)).astype(np.float16)
    wpT = np.ascontiguousarray(W_p.T)
    wp1 = wpT.astype(np.float16)
    wp2 = (wpT - wp1.astype(np.float32)).astype(np.float16)
    wa1 = W_a.astype(np.float16)
    vpr = np.ascontiguousarray(np.tile(V_p.reshape(1, PS), (128, 1)), dtype=np.float32)
    offs = np.tile(np.arange(-3, 4, dtype=np.float32).reshape(1, 1, NJ), (128, NTU, 1))
    offs = np.ascontiguousarray(offs.reshape(128, NWU))
    perm8 = np.zeros((128, 8, 128), dtype=np.float32)
    for w in range(8):
        for p in range(128):
            perm8[w * 16 + p % 16, w, p] = 1.0
    id128h = np.eye(128).astype(np.float16)

    consts = dict(wp1=wp1, wp2=wp2, wa1=wa1, vpr=vpr, offs=offs, perm8=perm8,
                  id128h=id128h)
    in_maps = []
    for k in range(NCORE):
        sl = slice(k * BPC, (k + 1) * BPC)
        m = dict(consts)
        m["qT16"] = np.ascontiguousarray(qT16[sl])
        m["cT1"] = np.ascontiguousarray(cT1[sl])
        m["cT2"] = np.ascontiguousarray(cT2[sl])
        in_maps.append(m)
    return in_maps


def kernel(q, c_t, W_a, W_p, V_p):
    global LAST_EXEC_NS, LAST_RESULT
    if "nc" not in _CACHE:
        _CACHE["nc"] = _build_nc()
    nc = _CACHE["nc"]
    in_maps = _host_prep(q, c_t, W_a, W_p, V_p)
    res = run_bass_kernel_spmd(nc, in_maps, core_ids=list(range(NCORE)))
    LAST_EXEC_NS = res.exec_time_ns
    LAST_RESULT = res
    outs = [res.results[k]["out"] for k in range(NCORE)]
    return np.concatenate(outs, axis=0).astype(np.float32)
